# revision 1
# baseline (speedup 1.0000x reference)
"""Connectome kernel (segment-mean -> Pearson Gram) for 8 TRN2 NeuronCores.

Strategy (pure data parallel, 2 samples per core):
  - Host marshalling: fold mask into parcellation; DROP background /
    masked-out pixels (~50% of V) entirely; sort surviving pixels by ROI
    and pack them into 128-pixel chunks (block B = ROIs 128..199 FIRST,
    then block A = ROIs 0..127; each block padded to a chunk boundary
    with label -1 slots). x is gathered into this packed order, cast
    fp16, laid out [p, chunk, sample, t] per core so each SBUF partition
    reads one contiguous HBM run per chunk-tile. Wire traffic per core:
    ~18.3MB (vs 73.7MB for fp32 all-pixels).
  - Device: stream chunk-tiles on the two HWDGE rings; onehots for ALL
    chunks are built in two batched DVE tensor_tensor ops (is_equal of
    broadcast iota vs broadcast labels); per chunk one PE matmul
    roiT[r, row] += onehot.T @ x_chunk (fp16 operands, fp32 PSUM).
    Block B accumulates first, so its Pearson-normalize chain runs on
    DVE while block A is still streaming.
  - Epilogue per core: the ROI-mean scaling and the +eps in the
    normalizer cancel in the Pearson Gram (normalize(c*s) == normalize(c)
    up to eps ~1e-8 relative), so work directly on the PSUM sums:
    mean + sumsq in two fused passes, 1/norm via reciprocal+sqrt,
    normalized rows emitted fp16, transpose + Gram on PE in fp16,
    write (2,200,200) fp32 conn to HBM.
  - Host: concat cores, extract upper triangle -> (16, 19900).
"""
import sys

sys.path.insert(0, "/opt/trn_rl_repo")

import numpy as np

import concourse.bass as bass
import concourse.tile as tile
from concourse import bacc, mybir
from concourse.bass_utils import run_bass_kernel_spmd

F32 = mybir.dt.float32
F16 = mybir.dt.float16

N, T, H, W = 16, 200, 144, 320
V = H * W                      # 46080
R = 200                        # ROIs
RA = 128                       # ROI block A width (ROIs 0..127)
RB = R - RA                    # ROI block B width (72; ROIs 128..199)
NCORES = 8
SPB = N // NCORES              # samples per core = 2
ROWS = SPB * T                 # 400
EPS = 1e-8                     # cancels in Gram; kept for reference only


def _tile_sizes(nch):
    """DMA tile schedule: small first tiles to fill the pipe fast, then 8s."""
    sizes, left = [], nch
    while left:
        ct = 4 if len(sizes) < 4 else 8
        ct = min(ct, left)
        sizes.append(ct)
        left -= ct
    return sizes

_cached = {}


def _bc3(ap2, ins_pos, n):
    """Insert a broadcast (stride 0, count n) dim into a 2D AP."""
    layout = [list(d) for d in ap2.ap]
    layout.insert(ins_pos, [0, n])
    return bass.AP(ap2.tensor, ap2.offset, layout)


def _split_st(ap2):
    """View a [P, SPB*T] AP as [P, SPB, T] (split the free dim)."""
    layout = [list(d) for d in ap2.ap]
    assert layout[-1][0] == 1 and layout[-1][1] == SPB * T
    layout = layout[:-1] + [[T, SPB], [1, T]]
    return bass.AP(ap2.tensor, ap2.offset, layout)


def _build_program(nA, nB):
    nch = nA + nB
    nc = bacc.Bacc("TRN2", target_bir_lowering=False, debug=False)

    x_d = nc.declare_dram_parameter("x", [128, nch, ROWS], F16, isOutput=False)
    labs_d = nc.declare_dram_parameter("labs", [128, nch], F16, isOutput=False)
    iota_d = nc.declare_dram_parameter("iota", [128, 128], F16, isOutput=False)
    i128_d = nc.declare_dram_parameter("i128", [128, 128], F16, isOutput=False)
    i72_d = nc.declare_dram_parameter("i72", [72, 72], F16, isOutput=False)
    out_d = nc.declare_dram_parameter("conn", [SPB, R, R], F32, isOutput=True)
    nrma_d = nc.declare_dram_parameter("nrm2a", [RA, SPB], F32, isOutput=True)
    nrmb_d = nc.declare_dram_parameter("nrm2b", [RB, SPB], F32, isOutput=True)

    tsizes = _tile_sizes(nch)

    with tile.TileContext(nc) as tc:
        with tc.tile_pool(name="consts", bufs=1) as consts, \
             tc.tile_pool(name="loads", bufs=3) as loads, \
             tc.tile_pool(name="ohp", bufs=1) as ohp, \
             tc.tile_pool(name="epi", bufs=1) as epi, \
             tc.tile_pool(name="psum", bufs=1, space="PSUM") as psum:

            labs_s = consts.tile([128, nch], F16)
            iota_s = consts.tile([128, 128], F16)
            i128_s = consts.tile([128, 128], F16)
            i72_s = consts.tile([72, 72], F16)
            # consts first on the sync HWDGE ring (tiny); x tile 0 goes on
            # the scalar ring concurrently, so nothing delays the stream.
            nc.sync.dma_start(labs_s[:], labs_d[:])
            nc.sync.dma_start(iota_s[:], iota_d[:])
            nc.sync.dma_start(i128_s[:], i128_d[:])
            nc.sync.dma_start(i72_s[:], i72_d[:])

            acc_a = psum.tile([RA, ROWS], F32, tag="acc_a", bufs=1)
            acc_b = psum.tile([RB, ROWS], F32, tag="acc_b", bufs=1)

            # PSUM tr tiles: [t-block, roi] transposed normalized rows.
            tr = {}
            for s in range(SPB):
                tr[("A", s)] = psum.tile([128, R], F16, tag="trA", bufs=2,
                                         name=f"trA_{s}")
                tr[("B", s)] = psum.tile([72, R], F16, tag="trB", bufs=2,
                                         name=f"trB_{s}")

            def emit_chain(blk, acc, P, nrm_d):
                """Demean per (roi, sample) directly from PSUM sums; emit
                per-sample fp16 centered rows + squared norms (normalization
                itself happens on the host as a rank-1 scaling of the Gram).
                """
                Ssb = epi.tile([P, ROWS], F32, tag=f"Ssb_{blk}")
                msum2 = epi.tile([P, SPB], F32, tag=f"ms_{blk}")
                m2 = epi.tile([P, SPB], F32, tag=f"m_{blk}")
                sq = epi.tile([P, ROWS], F32, tag=f"sq_{blk}")
                ssq2 = epi.tile([P, SPB], F32, tag=f"ssq_{blk}")
                t1 = epi.tile([P, SPB], F32, tag=f"t1_{blk}")
                nrm2 = epi.tile([P, SPB], F32, tag=f"n2_{blk}")
                # per-sample sums straight from PSUM (parallel with copy)
                nc.vector.tensor_reduce(msum2[:], _split_st(acc[:]),
                                        axis=mybir.AxisListType.X,
                                        op=mybir.AluOpType.add)
                nc.vector.tensor_scalar_mul(m2[:], msum2[:], 1.0 / T)
                cn = []
                for s in range(SPB):
                    n16 = epi.tile([P, T], F16, tag=f"n16_{blk}{s}")
                    nc.vector.tensor_scalar(n16[:], acc[:, bass.ts(s, T)],
                                            m2[:, s:s + 1], None,
                                            op0=mybir.AluOpType.subtract)
                    cn.append(n16)
                # squared norms: ssq - T*m^2 (host does 1/sqrt); off the
                # critical path to the Gram.
                nc.vector.tensor_copy(Ssb[:], acc[:])
                nc.vector.tensor_tensor(sq[:], acc[:], Ssb[:],
                                        op=mybir.AluOpType.mult)
                nc.vector.tensor_reduce(ssq2[:], _split_st(sq[:]),
                                        axis=mybir.AxisListType.X,
                                        op=mybir.AluOpType.add)
                nc.vector.tensor_mul(t1[:], m2[:], m2[:])
                nc.vector.scalar_tensor_tensor(
                    nrm2[:], t1[:], -float(T), ssq2[:],
                    op0=mybir.AluOpType.mult, op1=mybir.AluOpType.add)
                nc.scalar.dma_start(nrm_d[:], nrm2[:])
                return cn

            cn_b = None
            with nc.named_scope("main"):
                ch0 = 0
                for ti, ct in enumerate(tsizes):
                    ld = loads.tile([128, ct, ROWS], F16, tag=f"ld{ct}",
                                    bufs=(12 if ct == 8 else 4),
                                    name=f"ld_{ti}")
                    eng = nc.scalar if (ti % 2 == 0) else nc.sync
                    eng.dma_start(ld[:], x_d[:, ch0:ch0 + ct, :])

                    # batched per-tile onehot builds (DVE), one per block
                    # segment present in this tile
                    nb_i = max(0, min(nB, ch0 + ct) - ch0)       # B chunks
                    na_i = ct - nb_i                             # A chunks
                    ohB_t = ohA_t = None
                    if nb_i:
                        ohB_t = ohp.tile([128, nb_i, RB], F16,
                                         tag=f"ohB{nb_i}", bufs=4,
                                         name=f"ohB_{ti}")
                        nc.vector.tensor_tensor(
                            ohB_t[:], _bc3(iota_s[:, 0:RB], 1, nb_i),
                            _bc3(labs_s[:, ch0:ch0 + nb_i], 2, RB),
                            op=mybir.AluOpType.is_equal)
                    if na_i:
                        a0 = ch0 + nb_i
                        ohA_t = ohp.tile([128, na_i, RA], F16,
                                         tag=f"ohA{na_i}", bufs=4,
                                         name=f"ohA_{ti}")
                        nc.vector.tensor_tensor(
                            ohA_t[:], _bc3(iota_s[:, 0:RA], 1, na_i),
                            _bc3(labs_s[:, a0:a0 + na_i], 2, RA),
                            op=mybir.AluOpType.is_equal)

                    for j in range(ct):
                        cc = ch0 + j
                        if cc < nB:
                            acc, oh = acc_b, ohB_t[:, j, :]
                            start, stop = (cc == 0), (cc == nB - 1)
                        else:
                            acc, oh = acc_a, ohA_t[:, j - nb_i, :]
                            start, stop = (cc == nB), (cc == nch - 1)
                        nc.tensor.matmul(acc[:], oh, ld[:, j, :],
                                         start=start, stop=stop)
                    ch0 += ct

                    if ch0 - ct < nB <= ch0:
                        # block B complete: run its demean chain on DVE
                        # while block A still streams.
                        b_done_ti = ti
                        cn_b = emit_chain("b", acc_b, RB, nrmb_d)
                    if cn_b is not None and ti == b_done_ti + 4:
                        # B-sourced transposes, emitted a few tiles later so
                        # the chain has finished and PE's FIFO never blocks.
                        for s in range(SPB):
                            nc.tensor.transpose(tr[("A", s)][:, 128:200],
                                                cn_b[s][:, 0:128], i72_s[:])
                            nc.tensor.transpose(tr[("B", s)][:, 128:200],
                                                cn_b[s][:, 128:200], i72_s[:])

            with nc.named_scope("epilogue"):
                cn_a = emit_chain("a", acc_a, RA, nrma_d)
                for s in range(SPB):
                    trA, trB = tr[("A", s)], tr[("B", s)]
                    nc.tensor.transpose(trA[:, 0:128], cn_a[s][:, 0:128],
                                        i128_s[:])
                    nc.tensor.transpose(trB[:, 0:128], cn_a[s][:, 128:200],
                                        i128_s[:])
                    trA_sb = epi.tile([128, R], F16, name=f"trAs_{s}",
                                      tag="trAs", bufs=2)
                    trB_sb = epi.tile([72, R], F16, name=f"trBs_{s}",
                                      tag="trBs", bufs=2)
                    nc.vector.tensor_copy(trA_sb[:], trA[:])
                    nc.vector.tensor_copy(trB_sb[:], trB[:])

                    # Gram: conn = cn_t.T @ cn_t  (contraction over t, fp16)
                    cA = psum.tile([128, R], F32, tag="cA", bufs=1,
                                   name=f"cA_{s}")
                    cB = psum.tile([72, R], F32, tag="cB", bufs=1,
                                   name=f"cB_{s}")
                    nc.tensor.matmul(cA[:], trA_sb[:, 0:128], trA_sb[:],
                                     start=True, stop=False)
                    nc.tensor.matmul(cA[:], trB_sb[:, 0:128], trB_sb[:],
                                     start=False, stop=True)
                    nc.tensor.matmul(cB[:], trA_sb[:, 128:200], trA_sb[:],
                                     start=True, stop=False)
                    nc.tensor.matmul(cB[:], trB_sb[:, 128:200], trB_sb[:],
                                     start=False, stop=True)
                    cA_sb = epi.tile([128, R], F32, name=f"cAs_{s}", tag="cAs")
                    cB_sb = epi.tile([72, R], F32, name=f"cBs_{s}", tag="cBs")
                    nc.vector.tensor_copy(cA_sb[:], cA[:])
                    nc.vector.tensor_copy(cB_sb[:], cB[:])
                    nc.sync.dma_start(out_d[s, 0:128, :], cA_sb[:])
                    nc.scalar.dma_start(out_d[s, 128:200, :], cB_sb[:])

    nc.compile()
    return nc


def _get_program(nA, nB):
    key = (nA, nB)
    if key not in _cached:
        _cached[key] = _build_program(nA, nB)
    return _cached[key]


def marshal_inputs(x, parc, mask):
    """Host-side prep: packed ROI-sorted fp16 x + tiny derived constants."""
    parc_eff = np.where(np.asarray(mask), np.asarray(parc), 0).reshape(V)
    lab = parc_eff.astype(np.int64) - 1          # -1 = dropped
    counts = np.bincount(parc_eff.astype(np.int64), minlength=R + 1)[1:]

    order = np.argsort(lab, kind="stable")
    nbg = int((lab < 0).sum())
    sorted_idx = order[nbg:]                     # kept pixels, ROI-ascending
    cA = int(counts[0:RA].sum())
    cB = int(counts[RA:R].sum())
    nA = (cA + 127) // 128
    nB = (cB + 127) // 128

    # Block B (ROIs 128..199) first, then block A.
    gB = np.concatenate([sorted_idx[cA:],
                         np.zeros(nB * 128 - cB, dtype=np.int64)])
    gA = np.concatenate([sorted_idx[:cA],
                         np.zeros(nA * 128 - cA, dtype=np.int64)])
    g = np.concatenate([gB, gA])                 # (nch*128,) gather indices
    labB = np.concatenate([lab[sorted_idx[cA:]] - RA,
                           np.full(nB * 128 - cB, -1, dtype=np.int64)])
    labA = np.concatenate([lab[sorted_idx[:cA]],
                           np.full(nA * 128 - cA, -1, dtype=np.int64)])
    nch = nA + nB
    labs = np.concatenate([labB, labA]).astype(np.float16)
    labs = labs.reshape(nch, 128).T.copy()       # (128, nch)

    iota = np.broadcast_to(np.arange(128, dtype=np.float16), (128, 128)).copy()
    i128 = np.eye(128, dtype=np.float16)
    i72 = np.eye(72, dtype=np.float16)

    # (N,1,T,H,W) fp32 -> packed (core, 128, nch, SPB*T) fp16
    x16 = np.asarray(x, dtype=np.float32).reshape(N, T, V).astype(np.float16)
    xg = x16[:, :, g]                            # (N, T, nch*128)
    xg = xg.reshape(NCORES, SPB, T, nch, 128)
    xs = np.ascontiguousarray(xg.transpose(0, 4, 3, 1, 2))  # (8,128,nch,2,T)
    xs = xs.reshape(NCORES, 128, nch, ROWS)

    in_maps = []
    for c in range(NCORES):
        in_maps.append({
            "x": xs[c], "labs": labs, "iota": iota, "i128": i128, "i72": i72,
        })
    return in_maps, nA, nB


def kernel(x, parc, mask):
    in_maps, nA, nB = marshal_inputs(x, parc, mask)
    nc = _get_program(nA, nB)
    res = run_bass_kernel_spmd(nc, in_maps, core_ids=list(range(NCORES)))
    conn = np.concatenate([r["conn"] for r in res.results], axis=0)  # (16,200,200)
    # device emits the demeaned (unnormalized) Gram + squared norms;
    # normalization is a rank-1 row/col scaling: G = D Ghat D, D=1/||c||
    nrm2 = np.concatenate(
        [np.concatenate([r["nrm2a"], r["nrm2b"]], axis=0)[None]
         for r in res.results], axis=0)           # (8, 200, SPB)
    rinv = 1.0 / np.sqrt(nrm2)                    # (8, 200, SPB)
    rinv = rinv.transpose(0, 2, 1).reshape(N, R)  # (16, 200)
    conn = conn * rinv[:, :, None] * rinv[:, None, :]
    row, col = np.triu_indices(R, k=1)
    return np.ascontiguousarray(conn[:, row, col]).astype(np.float32)

